# revision 30
# baseline (speedup 1.0000x reference)
"""HGCN layer kernel for Trainium2, 8 NeuronCores, row-sharded SPMD.

Reference computation (N=6144, D=512):
    type_sum_a = adj_a @ x ; type_sum_b = adj_b @ x
    attn_a = sigmoid(cat[ts_a, x] @ Wa.T + ba) ; attn_b likewise
    h = x @ W_sa ; s_l = h @ a_sa[:512] ; s_r = h @ a_sa[512:]
    scores[i,j] = s_l[i] + s_r[j]
    e = adj_a * exp(-leaky_relu(scores, 0.01)) ; attn = e / (rowsum(e)+1e-5)
    x_a = attn @ h ; x_b = adj_b @ (x @ W_gcnb) + b_gcnb
    out = sigmoid(attn_a * x_a + attn_b * x_b)

Kernel strategy (per core, NL=768 local rows, global j order):
  - Phase A-local: h + stats for the LOCAL rows (f32r); h ships out via an
    AllGather that completes in the background.
  - Phase A-ext: xw = x @ W_gcnb and stats for ALL rows, replicated in bf16
    on every core (cheaper than waiting on a second gather: Phase B then
    depends only on on-chip data, and the h-gather hides under A-ext + B).
  - Gates reassociate (adj@x)@W1.T -> adj@(x@W1.T): ga/gb are M=1 row
    matmuls that stream the SAME adjacency tile already loaded for the
    main matmuls; rowsum(e) accumulates on the Vector engine.
  - e computed in transposed layout [j(part), i(free)]: lrelu+bias fused
    into one scalar Prelu, exp (bf16 out) on scalar, mask-mult on vector.
"""

import numpy as np
from contextlib import ExitStack

import concourse.bass as bass
import concourse.bacc as bacc
import concourse.mybir as mybir
import concourse.tile as tile

F32 = mybir.dt.float32
F32R = mybir.dt.float32r
BF16 = mybir.dt.bfloat16
F8E4 = mybir.dt.float8e4
AF = mybir.ActivationFunctionType
ALU = mybir.AluOpType

N_CORES = 8


def build_program(n, d, nl, ba, bb, dt_a=F32R, dt_bc=BF16):
    """Build the SPMD Bass program. Returns nc."""
    JT = n // 128   # j tiles (contraction/node axis), global order
    LT = nl // 128  # local row tiles
    KT = d // 128   # feature k tiles
    RL = d + 8      # r_loc cols:  W_sa | stats
    RE = d + 8      # r_ext cols:  W_gcnb | stats
    # stats cols: 0=s_l 1=s_r 2=zero 3=va 4=vb 5=wa2x 6=wb2x 7=pad

    nc = bacc.Bacc("TRN2", target_bir_lowering=False, debug=False,
                   num_devices=N_CORES)

    xtl_dram = nc.dram_tensor("xtl", [LT, 128, KT * 128], dt_a, kind="ExternalInput")
    xtf_dram = nc.dram_tensor("xtf", [JT, 128, KT * 128], dt_bc, kind="ExternalInput")
    rl_dram = nc.dram_tensor("rloc", [KT, 128, RL], dt_a, kind="ExternalInput")
    re_dram = nc.dram_tensor("rext", [KT, 128, RE], dt_bc, kind="ExternalInput")
    adjat_dram = nc.dram_tensor("adjat", [JT, 128, nl], dt_bc, kind="ExternalInput")
    adjbt_dram = nc.dram_tensor("adjbt", [JT, 128, nl], dt_bc, kind="ExternalInput")
    bbias_dram = nc.dram_tensor("bbias", [128, d], F32, kind="ExternalInput")
    ident_dram = nc.dram_tensor("ident", [128, 128], F32, kind="ExternalInput")
    out_dram = nc.dram_tensor("out", [nl, d], F32, kind="ExternalOutput")

    ag_in = nc.dram_tensor("ag_in", [LT, 128, d + 8], F8E4)
    ag_out = nc.dram_tensor("ag_out", [JT, 128, d + 8], F8E4,
                            addr_space="Shared")
    dmy_in = nc.dram_tensor("dmy_in", [1, 128, 2], F32)
    dmy_out = nc.dram_tensor("dmy_out", [N_CORES, 128, 2], F32,
                             addr_space="Shared")
    gate_dram = nc.dram_tensor("cc_gate", [1, 128, 2], F32)
    RG = [list(range(N_CORES))]

    def mm(out, lhsT, rhs, start, stop, skip_group_check=False):
        nc.tensor.matmul(out, lhsT, rhs, start=start, stop=stop,
                         skip_group_check=skip_group_check)

    with tile.TileContext(nc) as tc, ExitStack() as ctx:
        const = ctx.enter_context(tc.tile_pool(name="const", bufs=1))

        rl_sb = const.tile([128, KT, RL], dt_a, tag="rl")
        re_sb = const.tile([128, KT, RE], dt_bc, tag="re")
        h_sb = const.tile([128, JT // 2, 2, d], F8E4, tag="h")
        xw_sb = const.tile([128, JT * d], dt_bc, tag="xw")
        stats_sb = const.tile([128, JT * 8], F32, tag="stats")
        stats_r = const.tile([128, JT * 8], dt_bc, tag="statsr")
        stats_loc = const.tile([128, LT * 8], F32, tag="statsloc")
        slb_sb = const.tile([128, nl], F32, tag="slb")
        slb_bf = const.tile([128, nl], dt_bc, tag="slbbf")
        rs_acc = const.tile([128, nl], F32, tag="rs_acc")
        xb_sb = const.tile([128, LT * d], F32, tag="xb")
        t2_sb = const.tile([128, LT * d], F32, tag="t2")
        xa_sb = const.tile([128, LT * d], F32, tag="xa")
        bbias_sb = const.tile([128, d], F32, tag="bbias")
        ident_sb = const.tile([128, 128], F32, tag="ident")
        ones_f32 = const.tile([128, 1], F32, tag="ones_f32")
        ones_row = const.tile([1, 128], F32, tag="ones_r")
        neg1 = const.tile([128, 1], F32, tag="neg1")
        ba_sb = const.tile([128, 1], F32, tag="ba")
        bb_sb = const.tile([128, 1], F32, tag="bb")
        sl_row = const.tile([1, nl], F32, tag="sl_row")
        garow_sb = const.tile([1, nl], F32, tag="garow")
        gate_sb = const.tile([128, 4 * LT], F32, tag="gate")
        # gate_sb cols: [0:LT]=recip(rowsum), [LT:2LT]=sig_a, [2LT:3LT]=sig_b,
        # [3LT:4LT]=scratch

        chunks = [(o, min(512, nl - o)) for o in range(0, nl, 512)]
        NCH = len(chunks)
        dmy_sb = const.tile([128, 2], F32, tag="dmy")
        gsrc_sb = const.tile([128, 2], F8E4, tag="gsrc")

        # Warmup barrier: ncfw's first mesh starts ~65us after kernel start
        # no matter when triggered; pay that on a 1KB dummy while Phase A
        # computes, so the real gather (triggered later) starts instantly.
        nc.vector.memset(dmy_sb[:], 1.0)
        nc.gpsimd.dma_start(out=dmy_in[0], in_=dmy_sb[:])
        nc.gpsimd.collective_compute(
            "AllGather", mybir.AluOpType.bypass, replica_groups=RG,
            ins=[dmy_in[:].opt()], outs=[dmy_out[:].opt()])

        for k in range(KT):
            nc.sync.dma_start(out=rl_sb[:, k, :], in_=rl_dram[k])
            nc.scalar.dma_start(out=re_sb[:, k, :], in_=re_dram[k])
        nc.sync.dma_start(out=bbias_sb[:], in_=bbias_dram[:])
        nc.sync.dma_start(out=ident_sb[:], in_=ident_dram[:])
        nc.vector.memset(ones_f32[:], 1.0)
        nc.vector.memset(ones_row[:], 1.0)
        nc.vector.memset(neg1[:], -1.0)
        nc.vector.memset(ba_sb[:], float(ba))
        nc.vector.memset(bb_sb[:], float(bb))
        nc.vector.memset(rs_acc[:], 0.0)

        # ---- Phase A-local: h + stats for local rows (f32r) ----
        with tc.tile_pool(name="xtl_pool", bufs=2) as xtp, \
             tc.tile_pool(name="h_out", bufs=2) as hop, \
             tc.tile_pool(name="psAL", bufs=2, space="PSUM") as psA:
            for m in range(LT):
                xt_t = xtp.tile([128, KT * 128], dt_a, tag="xtl")
                nc.sync.dma_start(out=xt_t[:], in_=xtl_dram[m])
                ph = psA.tile([128, d], F32, tag="ph")
                ps = psA.tile([128, 8], F32, tag="ps")
                for k in range(KT):
                    lhsT = xt_t[:, k * 128:(k + 1) * 128]
                    st, sp = (k == 0), (k == KT - 1)
                    # tiny-N matmul first: its drain hides under the
                    # same-weights 512-wide stream that follows
                    mm(ps[:], lhsT, rl_sb[:, k, d:RL], st, sp)
                    mm(ph[:], lhsT, rl_sb[:, k, 0:d], st, sp)
                h_t = hop.tile([128, d], F8E4, tag="ht")
                nc.scalar.copy(h_t[:], ph[:])
                nc.vector.tensor_copy(stats_loc[:, m * 8:(m + 1) * 8], ps[:])
                nc.gpsimd.dma_start(out=ag_in[m, :, 0:d], in_=h_t[:])

        # ---- Phase A2: build SL broadcast [128, nl] from local s_l ----
        with tc.tile_pool(name="psA2", bufs=1, space="PSUM") as psA2:
            ptrs = [psA2.tile([1, w], F32, tag=f"psl{ci}", name=f"psl{ci}")
                    for ci, (o, w) in enumerate(chunks)]
            for t in range(LT):
                ci, off = divmod(t * 128, 512)
                mm(ptrs[ci][0:1, off:off + 128],
                   stats_loc[:, t * 8:t * 8 + 1], ident_sb[:], True, True)
            for ci, (o, w) in enumerate(chunks):
                nc.vector.tensor_copy(sl_row[0:1, o:o + w], ptrs[ci][0:1, :])
            for ci, (o, w) in enumerate(chunks):
                pb = psA2.tile([128, w], F32, tag="pslb")
                mm(pb[:], ones_row[:], sl_row[0:1, o:o + w], True, True)
                nc.vector.tensor_copy(slb_sb[:, o:o + w], pb[:])
                nc.vector.tensor_copy(slb_bf[:, o:o + w], pb[:])

        # ---- Phase A-ext: xw + stats for ALL rows, replicated (bf16) ----
        with tc.tile_pool(name="xtf_pool", bufs=3) as xtp, \
             tc.tile_pool(name="psAE", bufs=3, space="PSUM") as psA:
            for m in range(JT):
                xt_t = xtp.tile([128, KT * 128], dt_bc, tag="xtf")
                nc.sync.dma_start(out=xt_t[:], in_=xtf_dram[m])
                pw = psA.tile([128, d], F32, tag="pw")
                ps = psA.tile([128, 8], F32, tag="ps2")
                for k in range(KT):
                    lhsT = xt_t[:, k * 128:(k + 1) * 128]
                    st, sp = (k == 0), (k == KT - 1)
                    mm(ps[:], lhsT, re_sb[:, k, d:RE], st, sp)
                    mm(pw[:], lhsT, re_sb[:, k, 0:d], st, sp)
                nc.vector.tensor_copy(xw_sb[:, m * d:(m + 1) * d], pw[:])
                nc.vector.tensor_copy(stats_sb[:, m * 8:(m + 1) * 8], ps[:])
                nc.vector.tensor_copy(stats_r[:, m * 8:(m + 1) * 8], ps[:])

        # ---- AllGather h, gated to run after A-ext: the pad-column write
        # into ag_in is a true data dependency of the collective (Tile
        # hoists collectives past mere queue ordering), so the transfer's
        # DMA-engine takeover lands in Phase B where prefetch rides it. ----
        nc.vector.tensor_copy(gsrc_sb[:],
                              stats_r[:, (JT - 1) * 8:(JT - 1) * 8 + 2])
        nc.gpsimd.dma_start(out=ag_in[0, :, d:d + 2], in_=gsrc_sb[:])
        nc.gpsimd.collective_compute(
            "AllGather", mybir.AluOpType.bypass, replica_groups=RG,
            ins=[ag_in[:].opt()], outs=[ag_out[:].opt()])
        for j in range(JT):
            nc.scalar.dma_start(out=h_sb[:, j // 2, j % 2, :],
                                in_=ag_out[j, :, 0:d])

        # ---- Phase B: x_b = adj_b @ xW ; gb rides the loaded adj weights ----
        with tc.tile_pool(name="adjB", bufs=14) as adjp, \
             tc.tile_pool(name="psB", bufs=1, space="PSUM") as psB:
            pb_acc = [psB.tile([128, d], F32, tag=f"pb{i}", name=f"pb{i}")
                      for i in range(LT)]
            pgbT = psB.tile([128, d], F32, tag="pgbT")
            for j in range(JT):
                at = adjp.tile([128, nl], dt_bc, tag="adj")
                nc.sync.dma_start(out=at[:], in_=adjbt_dram[j])
                xw_j = xw_sb[:, j * d:(j + 1) * d]
                vb_j = stats_r[:, j * 8 + 4:j * 8 + 5]
                st, sp = (j == 0), (j == JT - 1)
                for i in range(LT):
                    ai = at[:, i * 128:(i + 1) * 128]
                    mm(pgbT[:, i:i + 1], ai, vb_j, st and i == 0,
                       sp and i == LT - 1, skip_group_check=True)
                    mm(pb_acc[i][:], ai, xw_j, st, sp)
            for i in range(LT):
                nc.scalar.copy(xb_sb[:, i * d:(i + 1) * d], pb_acc[i][:])
            nc.vector.tensor_copy(gate_sb[:, 2 * LT:3 * LT], pgbT[:, 0:LT])
        for i in range(LT):
            # sig_b = sigmoid(gb + wb2x + bb); t2 = (x_b + b_gcnb) * sig_b
            nc.vector.tensor_tensor(gate_sb[:, 2 * LT + i:2 * LT + i + 1],
                                    gate_sb[:, 2 * LT + i:2 * LT + i + 1],
                                    stats_loc[:, i * 8 + 6:i * 8 + 7],
                                    op=ALU.add)
            nc.scalar.activation(gate_sb[:, 2 * LT + i:2 * LT + i + 1],
                                 gate_sb[:, 2 * LT + i:2 * LT + i + 1],
                                 AF.Sigmoid, bias=bb_sb[:])
            nc.vector.tensor_tensor(t2_sb[:, i * d:(i + 1) * d],
                                    xb_sb[:, i * d:(i + 1) * d],
                                    bbias_sb[:], op=ALU.add)
            nc.vector.tensor_scalar_mul(t2_sb[:, i * d:(i + 1) * d],
                                        t2_sb[:, i * d:(i + 1) * d],
                                        gate_sb[:, 2 * LT + i:2 * LT + i + 1])

        # ---- Phase C: e = adj_a * exp(-lrelu(s)); x_a = e^T.T @ h ----
        with tc.tile_pool(name="adjC", bufs=5) as adjp, \
             tc.tile_pool(name="ewC", bufs=3) as ewp, \
             tc.tile_pool(name="psC", bufs=1, space="PSUM") as psC:
            pc_acc = [psC.tile([128, d], F32, tag=f"pc{i}", name=f"pc{i}")
                      for i in range(LT)]
            pga = [psC.tile([1, w], F32, tag=f"pga{ci}", name=f"pga{ci}")
                   for ci, (o, w) in enumerate(chunks)]
            e2 = None
            for j in range(JT):
                at = adjp.tile([128, nl], dt_bc, tag="adj")
                nc.sync.dma_start(out=at[:], in_=adjat_dram[j])
                s_r = stats_sb[:, j * 8 + 1:j * 8 + 2]
                m_t = ewp.tile([128, nl], dt_bc, tag="m")
                nc.vector.tensor_scalar_add(m_t[:], slb_bf[:], s_r)
                nc.vector.scalar_tensor_tensor(m_t[:], m_t[:], 0.01, m_t[:],
                                               op0=ALU.mult, op1=ALU.max)
                w_t = ewp.tile([128, nl], dt_bc, tag="w")
                nc.scalar.activation(w_t[:], m_t[:], AF.Exp, scale=neg1[:])
                if j % 2 == 0:
                    e2 = ewp.tile([128, 2, nl], F8E4, tag="e2")
                e_t = e2[:, j % 2, :]
                nc.gpsimd.tensor_tensor(e_t, w_t[:], at[:], op=ALU.mult)
                nc.vector.tensor_tensor(rs_acc[:], rs_acc[:], e_t, op=ALU.add)
                va_j = stats_r[:, j * 8 + 3:j * 8 + 4]
                st, sp = (j == 0), (j == JT - 1)
                for ci, (o, w) in enumerate(chunks):
                    mm(pga[ci][:], va_j, at[:, o:o + w], st, sp)
                if j % 2 == 1:
                    jp = j // 2
                    for i in range(LT):
                        nc.tensor.matmul(
                            pc_acc[i][:], e2[:, :, i * 128:(i + 1) * 128],
                            h_sb[:, jp], start=(jp == 0),
                            stop=(jp == JT // 2 - 1),
                            perf_mode=mybir.MatmulPerfMode.DoubleRow)
            for i in range(LT):
                nc.scalar.copy(xa_sb[:, i * d:(i + 1) * d], pc_acc[i][:])
            for ci, (o, w) in enumerate(chunks):
                nc.vector.tensor_copy(garow_sb[0:1, o:o + w], pga[ci][0:1, :])

        # ---- Phase D: transposes, gates, combine ----
        with tc.tile_pool(name="psD", bufs=1, space="PSUM") as psD, \
             tc.tile_pool(name="outD", bufs=2) as outp:
            pT = psD.tile([128, d], F32, tag="pT")
            # rowsum partition-reduce -> columns (cols 0:LT of pT)
            for i in range(LT):
                mm(pT[:, i:i + 1], rs_acc[:, i * 128:(i + 1) * 128],
                   ones_f32[:], i == 0, False, skip_group_check=True)
            # ga row -> columns (cols LT:2LT); gb arrives pre-columned
            for i in range(LT):
                mm(pT[:, LT + i:LT + i + 1],
                   garow_sb[0:1, i * 128:(i + 1) * 128],
                   ones_row[0:1, 0:1], False, i == LT - 1,
                   skip_group_check=True)
            for i in range(LT):
                # recip(rowsum + 1e-5)
                nc.vector.tensor_scalar_add(gate_sb[:, 3 * LT + i:3 * LT + i + 1],
                                            pT[:, i:i + 1], 1e-5)
                nc.vector.reciprocal(gate_sb[:, i:i + 1],
                                     gate_sb[:, 3 * LT + i:3 * LT + i + 1])
                # sig_a = sigmoid(ga + wa2x + ba)
                nc.vector.tensor_tensor(gate_sb[:, LT + i:LT + i + 1],
                                        pT[:, LT + i:LT + i + 1],
                                        stats_loc[:, i * 8 + 5:i * 8 + 6],
                                        op=ALU.add)
                nc.scalar.activation(gate_sb[:, LT + i:LT + i + 1],
                                     gate_sb[:, LT + i:LT + i + 1],
                                     AF.Sigmoid, bias=ba_sb[:])
            for i in range(LT):
                u_t = outp.tile([128, d], F32, tag="u")
                # u = sig_a * (x_a_raw * recip)
                nc.vector.tensor_scalar(u_t[:], xa_sb[:, i * d:(i + 1) * d],
                                        gate_sb[:, i:i + 1],
                                        gate_sb[:, LT + i:LT + i + 1],
                                        op0=ALU.mult, op1=ALU.mult)
                t_t = outp.tile([128, d], F32, tag="t")
                # y = sigmoid(t2 + u)
                nc.vector.tensor_tensor(t_t[:], t2_sb[:, i * d:(i + 1) * d],
                                        u_t[:], op=ALU.add)
                y_t = outp.tile([128, d], F32, tag="y")
                nc.scalar.activation(y_t[:], t_t[:], AF.Sigmoid)
                nc.sync.dma_start(out=out_dram[i * 128:(i + 1) * 128, :],
                                  in_=y_t[:])

    nc.compile()
    return nc


def make_r_matrices(W_sa, a_sa, W_gcnb, Wa, Wb, d):
    cols = np.zeros((d, 8), dtype=np.float32)
    cols[:, 0] = W_sa @ a_sa[0, :d]
    cols[:, 1] = W_sa @ a_sa[0, d:]
    # col 2 stays zero
    cols[:, 3] = Wa[0, :d]
    cols[:, 4] = Wb[0, :d]
    cols[:, 5] = Wa[0, d:]
    cols[:, 6] = Wb[0, d:]
    r_loc = np.ascontiguousarray(np.concatenate([W_sa, cols], axis=1))
    r_ext = np.ascontiguousarray(np.concatenate([W_gcnb, cols], axis=1))
    return r_loc.astype(np.float32), r_ext.astype(np.float32)


def _pack_lhsT(xm, KT):
    """[M*128, d] row-block -> [M, 128, KT*128] with tile (m,k) = block^T.

    Element (m, p, k*128+r) = xm[m*128+r, k*128+p], so a single [128, KT*128]
    DMA per m-tile lands k-blocks side by side in SBUF columns.
    """
    M = xm.shape[0] // 128
    return np.ascontiguousarray(
        xm.reshape(M, 128, KT, 128).transpose(0, 3, 2, 1).reshape(
            M, 128, KT * 128))


def make_shared_inputs(x, r_loc, r_ext, b_gcnb, n, d, np_bc):
    JT, KT = n // 128, d // 128
    xtf = _pack_lhsT(x, KT)
    return {
        "xtf": xtf.astype(np_bc),
        "rloc": r_loc.reshape(KT, 128, d + 8).astype(np.float32),
        "rext": r_ext.reshape(KT, 128, d + 8).astype(np_bc),
        "bbias": np.ascontiguousarray(
            np.broadcast_to(b_gcnb, (128, d))).astype(np.float32),
        "ident": np.eye(128, dtype=np.float32),
    }


def make_core_inputs(x, adj_a, adj_b, shared, n, d, nl, core, np_bc):
    JT, KT, LT = n // 128, d // 128, nl // 128
    rows = np.arange(core * nl, (core + 1) * nl)
    xtl = _pack_lhsT(x[rows], KT)
    adjat = np.ascontiguousarray(adj_a[rows].T).reshape(JT, 128, nl)
    adjbt = np.ascontiguousarray(adj_b[rows].T).reshape(JT, 128, nl)
    return {
        "xtl": xtl.astype(np.float32),
        "adjat": adjat.astype(np_bc),
        "adjbt": adjbt.astype(np_bc),
        **shared,
    }


_CACHE = {}


def _install_ntff_hook():
    """Dev-only: register the axon NTFF profile hook so trace=True works."""
    import sys
    import types
    try:
        from antenv import axon_hooks  # noqa: F401
        return
    except ImportError:
        pass
    import antenv
    mod = types.ModuleType("antenv.axon_hooks")
    _h = [None]
    mod.get_axon_ntff_profile_hook = lambda: _h[0]
    mod.set_axon_ntff_profile_hook = lambda hook: _h.__setitem__(0, hook)
    sys.modules["antenv.axon_hooks"] = mod
    antenv.axon_hooks = mod
    from trn_agent_boot.trn_boot import _ntff_profile_via_ctypes
    mod.set_axon_ntff_profile_hook(
        _ntff_profile_via_ctypes("/opt/axon/libaxon_pjrt.so"))


def kernel(x, adj_a, adj_b, W_sa, a_sa, W_gcnb, b_gcnb, Wa, ba, Wb, bb,
           _trace=False, _trace_kwargs=None):
    from concourse.bass_utils import run_bass_kernel_spmd
    if _trace:
        _install_ntff_hook()

    n, d = x.shape
    nl = n // N_CORES
    r_loc, r_ext = make_r_matrices(W_sa, a_sa, W_gcnb, Wa, Wb, d)

    key = (n, d, nl, float(ba[0]), float(bb[0]))
    if key not in _CACHE:
        _CACHE[key] = build_program(n, d, nl, float(ba[0]), float(bb[0]))
    nc = _CACHE[key]

    import ml_dtypes
    np_bc = ml_dtypes.bfloat16
    shared = make_shared_inputs(x, r_loc, r_ext, b_gcnb, n, d, np_bc)
    in_maps = [make_core_inputs(x, adj_a, adj_b, shared, n, d, nl, c, np_bc)
               for c in range(N_CORES)]
    res = run_bass_kernel_spmd(nc, in_maps, list(range(N_CORES)),
                               trace=_trace, **(_trace_kwargs or {}))
    out = np.empty((n, d), dtype=np.float32)
    for c in range(N_CORES):
        out[c * nl:(c + 1) * nl] = res.results[c]["out"]
    if _trace:
        kernel._last_results = res
    return out


# revision 32
# speedup vs baseline: 1.4881x; 1.4881x over previous
"""HGCN layer kernel for Trainium2, 8 NeuronCores, row-sharded SPMD.

Reference computation (N=6144, D=512):
    type_sum_a = adj_a @ x ; type_sum_b = adj_b @ x
    attn_a = sigmoid(cat[ts_a, x] @ Wa.T + ba) ; attn_b likewise
    h = x @ W_sa ; s_l = h @ a_sa[:512] ; s_r = h @ a_sa[512:]
    scores[i,j] = s_l[i] + s_r[j]
    e = adj_a * exp(-leaky_relu(scores, 0.01)) ; attn = e / (rowsum(e)+1e-5)
    x_a = attn @ h ; x_b = adj_b @ (x @ W_gcnb) + b_gcnb
    out = sigmoid(attn_a * x_a + attn_b * x_b)

Kernel strategy (per core, NL=768 local rows, global j order):
  - Phase A-local: h + stats for the LOCAL rows (f32r); h ships out via an
    AllGather that completes in the background.
  - Phase A-ext: xw = x @ W_gcnb and stats for ALL rows, replicated in bf16
    on every core (cheaper than waiting on a second gather: Phase B then
    depends only on on-chip data, and the h-gather hides under A-ext + B).
  - Gates reassociate (adj@x)@W1.T -> adj@(x@W1.T): ga/gb are M=1 row
    matmuls that stream the SAME adjacency tile already loaded for the
    main matmuls; rowsum(e) accumulates on the Vector engine.
  - e computed in transposed layout [j(part), i(free)]: lrelu+bias fused
    into one scalar Prelu, exp (bf16 out) on scalar, mask-mult on vector.
"""

import numpy as np
from contextlib import ExitStack

import concourse.bass as bass
import concourse.bacc as bacc
import concourse.mybir as mybir
import concourse.tile as tile

F32 = mybir.dt.float32
F32R = mybir.dt.float32r
BF16 = mybir.dt.bfloat16
F8E4 = mybir.dt.float8e4
AF = mybir.ActivationFunctionType
ALU = mybir.AluOpType

N_CORES = 8


def build_program(n, d, nl, ba, bb, dt_a=F32R, dt_bc=BF16):
    """Build the SPMD Bass program. Returns nc."""
    JT = n // 128   # j tiles (contraction/node axis), global order
    LT = nl // 128  # local row tiles
    KT = d // 128   # feature k tiles
    RL = d + 8      # r_loc cols:  W_sa | stats
    RE = d + 8      # r_ext cols:  W_gcnb | stats
    # stats cols: 0=s_l 1=s_r 2=zero 3=va 4=vb 5=wa2x 6=wb2x 7=pad

    nc = bacc.Bacc("TRN2", target_bir_lowering=False, debug=False,
                   num_devices=N_CORES)

    xtl_dram = nc.dram_tensor("xtl", [LT, 128, KT * 128], dt_a, kind="ExternalInput")
    xtf_dram = nc.dram_tensor("xtf", [JT, 128, KT * 128], dt_bc, kind="ExternalInput")
    rl_dram = nc.dram_tensor("rloc", [KT, 128, RL], dt_a, kind="ExternalInput")
    re_dram = nc.dram_tensor("rext", [KT, 128, RE], dt_bc, kind="ExternalInput")
    adjat_dram = nc.dram_tensor("adjat", [JT, 128, nl], dt_bc, kind="ExternalInput")
    adjbt_dram = nc.dram_tensor("adjbt", [JT, 128, nl], dt_bc, kind="ExternalInput")
    bbias_dram = nc.dram_tensor("bbias", [128, d], F32, kind="ExternalInput")
    ident_dram = nc.dram_tensor("ident", [128, 128], F32, kind="ExternalInput")
    out_dram = nc.dram_tensor("out", [nl, d], F32, kind="ExternalOutput")

    ag_in = nc.dram_tensor("ag_in", [LT, 128, d + 8], F8E4)
    ag_out = nc.dram_tensor("ag_out", [JT, 128, d + 8], F8E4,
                            addr_space="Shared")
    dmy_in = nc.dram_tensor("dmy_in", [1, 128, 2], F32)
    dmy_out = nc.dram_tensor("dmy_out", [N_CORES, 128, 2], F32,
                             addr_space="Shared")
    gate_dram = nc.dram_tensor("cc_gate", [1, 128, 2], F32)
    RG = [list(range(N_CORES))]

    def mm(out, lhsT, rhs, start, stop, skip_group_check=False):
        nc.tensor.matmul(out, lhsT, rhs, start=start, stop=stop,
                         skip_group_check=skip_group_check)

    with tile.TileContext(nc) as tc, ExitStack() as ctx:
        const = ctx.enter_context(tc.tile_pool(name="const", bufs=1))

        rl_sb = const.tile([128, KT, RL], dt_a, tag="rl")
        re_sb = const.tile([128, KT, RE], dt_bc, tag="re")
        h_sb = const.tile([128, JT // 2, 2, d], F8E4, tag="h")
        xw_sb = const.tile([128, JT * d], dt_bc, tag="xw")
        stats_sb = const.tile([128, JT * 8], F32, tag="stats")
        stats_r = const.tile([128, JT * 8], dt_bc, tag="statsr")
        stats_loc = const.tile([128, LT * 8], F32, tag="statsloc")
        slb_sb = const.tile([128, nl], F32, tag="slb")
        rs_acc = const.tile([128, nl], F32, tag="rs_acc")
        xb_sb = const.tile([128, LT * d], F32, tag="xb")
        t2_sb = const.tile([128, LT * d], F32, tag="t2")
        xa_sb = const.tile([128, LT * d], F32, tag="xa")
        bbias_sb = const.tile([128, d], F32, tag="bbias")
        ident_sb = const.tile([128, 128], F32, tag="ident")
        ones_f32 = const.tile([128, 1], F32, tag="ones_f32")
        ones_row = const.tile([1, 128], F32, tag="ones_r")
        neg1 = const.tile([128, 1], F32, tag="neg1")
        ba_sb = const.tile([128, 1], F32, tag="ba")
        bb_sb = const.tile([128, 1], F32, tag="bb")
        sl_row = const.tile([1, nl], F32, tag="sl_row")
        garow_sb = const.tile([1, nl], F32, tag="garow")
        gate_sb = const.tile([128, 4 * LT], F32, tag="gate")
        # gate_sb cols: [0:LT]=recip(rowsum), [LT:2LT]=sig_a, [2LT:3LT]=sig_b,
        # [3LT:4LT]=scratch

        chunks = [(o, min(512, nl - o)) for o in range(0, nl, 512)]
        NCH = len(chunks)
        dmy_sb = const.tile([128, 2], F32, tag="dmy")
        gsrc_sb = const.tile([128, 2], F8E4, tag="gsrc")

        # Warmup barrier: ncfw's first mesh starts ~65us after kernel start
        # no matter when triggered; pay that on a 1KB dummy while Phase A
        # computes, so the real gather (triggered later) starts instantly.
        nc.vector.memset(dmy_sb[:], 1.0)
        nc.gpsimd.dma_start(out=dmy_in[0], in_=dmy_sb[:])
        nc.gpsimd.collective_compute(
            "AllGather", mybir.AluOpType.bypass, replica_groups=RG,
            ins=[dmy_in[:].opt()], outs=[dmy_out[:].opt()])

        for k in range(KT):
            nc.sync.dma_start(out=rl_sb[:, k, :], in_=rl_dram[k])
            nc.scalar.dma_start(out=re_sb[:, k, :], in_=re_dram[k])
        nc.sync.dma_start(out=bbias_sb[:], in_=bbias_dram[:])
        nc.sync.dma_start(out=ident_sb[:], in_=ident_dram[:])
        nc.vector.memset(ones_f32[:], 1.0)
        nc.vector.memset(ones_row[:], 1.0)
        nc.vector.memset(neg1[:], -1.0)
        nc.vector.memset(ba_sb[:], float(ba))
        nc.vector.memset(bb_sb[:], float(bb))
        nc.vector.memset(rs_acc[:], 0.0)

        # ---- Phase A-local: h + stats for local rows (f32r) ----
        with tc.tile_pool(name="xtl_pool", bufs=3) as xtp, \
             tc.tile_pool(name="h_out", bufs=2) as hop, \
             tc.tile_pool(name="psAL", bufs=2, space="PSUM") as psA:
            for m in range(LT):
                xt_t = xtp.tile([128, KT * 128], dt_a, tag="xtl")
                nc.sync.dma_start(out=xt_t[:], in_=xtl_dram[m])
                ph = psA.tile([128, d], F32, tag="ph")
                ps = psA.tile([128, 8], F32, tag="ps")
                for k in range(KT):
                    lhsT = xt_t[:, k * 128:(k + 1) * 128]
                    st, sp = (k == 0), (k == KT - 1)
                    # tiny-N matmul first: its drain hides under the
                    # same-weights 512-wide stream that follows
                    mm(ps[:], lhsT, rl_sb[:, k, d:RL], st, sp)
                    mm(ph[:], lhsT, rl_sb[:, k, 0:d], st, sp)
                h_t = hop.tile([128, d], F8E4, tag="ht")
                nc.scalar.copy(h_t[:], ph[:])
                nc.vector.tensor_copy(stats_loc[:, m * 8:(m + 1) * 8], ps[:])
                nc.gpsimd.dma_start(out=ag_in[m, :, 0:d], in_=h_t[:])

        # ---- Phase A2: build SL broadcast [128, nl] from local s_l ----
        with tc.tile_pool(name="psA2", bufs=1, space="PSUM") as psA2:
            ptrs = [psA2.tile([1, w], F32, tag=f"psl{ci}", name=f"psl{ci}")
                    for ci, (o, w) in enumerate(chunks)]
            for t in range(LT):
                ci, off = divmod(t * 128, 512)
                mm(ptrs[ci][0:1, off:off + 128],
                   stats_loc[:, t * 8:t * 8 + 1], ident_sb[:], True, True)
            for ci, (o, w) in enumerate(chunks):
                nc.vector.tensor_copy(sl_row[0:1, o:o + w], ptrs[ci][0:1, :])
            for ci, (o, w) in enumerate(chunks):
                pb = psA2.tile([128, w], F32, tag="pslb")
                mm(pb[:], ones_row[:], sl_row[0:1, o:o + w], True, True)
                nc.vector.tensor_copy(slb_sb[:, o:o + w], pb[:])

        # ---- Phase A-ext: xw + stats for ALL rows, replicated (bf16) ----
        with tc.tile_pool(name="xtf_pool", bufs=5) as xtp, \
             tc.tile_pool(name="psAE", bufs=3, space="PSUM") as psA:
            for m in range(JT):
                xt_t = xtp.tile([128, KT * 128], dt_bc, tag="xtf")
                nc.sync.dma_start(out=xt_t[:], in_=xtf_dram[m])
                pw = psA.tile([128, d], F32, tag="pw")
                ps = psA.tile([128, 8], F32, tag="ps2")
                for k in range(KT):
                    lhsT = xt_t[:, k * 128:(k + 1) * 128]
                    st, sp = (k == 0), (k == KT - 1)
                    mm(ps[:], lhsT, re_sb[:, k, d:RE], st, sp)
                    mm(pw[:], lhsT, re_sb[:, k, 0:d], st, sp)
                nc.vector.tensor_copy(xw_sb[:, m * d:(m + 1) * d], pw[:])
                nc.vector.tensor_copy(stats_sb[:, m * 8:(m + 1) * 8], ps[:])
                nc.vector.tensor_copy(stats_r[:, m * 8:(m + 1) * 8], ps[:])

        # ---- AllGather h, gated to run after A-ext: the pad-column write
        # into ag_in is a true data dependency of the collective (Tile
        # hoists collectives past mere queue ordering), so the transfer's
        # DMA-engine takeover lands in Phase B where prefetch rides it. ----
        nc.vector.tensor_copy(gsrc_sb[:],
                              stats_r[:, (JT - 1) * 8:(JT - 1) * 8 + 2])
        nc.gpsimd.dma_start(out=ag_in[0, :, d:d + 2], in_=gsrc_sb[:])
        nc.gpsimd.collective_compute(
            "AllGather", mybir.AluOpType.bypass, replica_groups=RG,
            ins=[ag_in[:].opt()], outs=[ag_out[:].opt()])
        for j in range(JT):
            nc.scalar.dma_start(out=h_sb[:, j // 2, j % 2, :],
                                in_=ag_out[j, :, 0:d])

        # ---- Phase B: x_b = adj_b @ xW ; gb rides the loaded adj weights ----
        with tc.tile_pool(name="adjB", bufs=20) as adjp, \
             tc.tile_pool(name="psB", bufs=1, space="PSUM") as psB:
            pb_acc = [psB.tile([128, d], F32, tag=f"pb{i}", name=f"pb{i}")
                      for i in range(LT)]
            pgbT = psB.tile([128, d], F32, tag="pgbT")
            for j in range(JT):
                at = adjp.tile([128, nl], dt_bc, tag="adj")
                nc.sync.dma_start(out=at[:], in_=adjbt_dram[j])
                xw_j = xw_sb[:, j * d:(j + 1) * d]
                vb_j = stats_r[:, j * 8 + 4:j * 8 + 5]
                st, sp = (j == 0), (j == JT - 1)
                for i in range(LT):
                    ai = at[:, i * 128:(i + 1) * 128]
                    mm(pgbT[:, i:i + 1], ai, vb_j, st and i == 0,
                       sp and i == LT - 1, skip_group_check=True)
                    mm(pb_acc[i][:], ai, xw_j, st, sp)
            for i in range(LT):
                nc.scalar.copy(xb_sb[:, i * d:(i + 1) * d], pb_acc[i][:])
            nc.vector.tensor_copy(gate_sb[:, 2 * LT:3 * LT], pgbT[:, 0:LT])
        for i in range(LT):
            # sig_b = sigmoid(gb + wb2x + bb); t2 = (x_b + b_gcnb) * sig_b
            nc.vector.tensor_tensor(gate_sb[:, 2 * LT + i:2 * LT + i + 1],
                                    gate_sb[:, 2 * LT + i:2 * LT + i + 1],
                                    stats_loc[:, i * 8 + 6:i * 8 + 7],
                                    op=ALU.add)
            nc.scalar.activation(gate_sb[:, 2 * LT + i:2 * LT + i + 1],
                                 gate_sb[:, 2 * LT + i:2 * LT + i + 1],
                                 AF.Sigmoid, bias=bb_sb[:])
            nc.vector.tensor_tensor(t2_sb[:, i * d:(i + 1) * d],
                                    xb_sb[:, i * d:(i + 1) * d],
                                    bbias_sb[:], op=ALU.add)
            nc.vector.tensor_scalar_mul(t2_sb[:, i * d:(i + 1) * d],
                                        t2_sb[:, i * d:(i + 1) * d],
                                        gate_sb[:, 2 * LT + i:2 * LT + i + 1])

        # ---- Phase C: e = adj_a * exp(-lrelu(s)); x_a = e^T.T @ h ----
        with tc.tile_pool(name="adjC", bufs=8) as adjp, \
             tc.tile_pool(name="ewC", bufs=4) as ewp, \
             tc.tile_pool(name="psC", bufs=1, space="PSUM") as psC:
            pc_acc = [psC.tile([128, d], F32, tag=f"pc{i}", name=f"pc{i}")
                      for i in range(LT)]
            pga = [psC.tile([1, w], F32, tag=f"pga{ci}", name=f"pga{ci}")
                   for ci, (o, w) in enumerate(chunks)]
            e2 = None
            for j in range(JT):
                at = adjp.tile([128, nl], dt_bc, tag="adj")
                nc.sync.dma_start(out=at[:], in_=adjat_dram[j])
                s_r = stats_sb[:, j * 8 + 1:j * 8 + 2]
                m_t = ewp.tile([128, nl], F32, tag="m")
                nc.scalar.activation(m_t[:], slb_sb[:], AF.Prelu,
                                     bias=s_r, alpha=0.01)
                w_t = ewp.tile([128, nl], dt_bc, tag="w")
                nc.scalar.activation(w_t[:], m_t[:], AF.Exp, scale=neg1[:])
                if j % 2 == 0:
                    e2 = ewp.tile([128, 2, nl], F8E4, tag="e2")
                e_t = e2[:, j % 2, :]
                nc.vector.tensor_tensor(e_t, w_t[:], at[:], op=ALU.mult)
                nc.vector.tensor_tensor(rs_acc[:], rs_acc[:], e_t, op=ALU.add)
                va_j = stats_r[:, j * 8 + 3:j * 8 + 4]
                st, sp = (j == 0), (j == JT - 1)
                for ci, (o, w) in enumerate(chunks):
                    mm(pga[ci][:], va_j, at[:, o:o + w], st, sp)
                if j % 2 == 1:
                    jp = j // 2
                    for i in range(LT):
                        nc.tensor.matmul(
                            pc_acc[i][:], e2[:, :, i * 128:(i + 1) * 128],
                            h_sb[:, jp], start=(jp == 0),
                            stop=(jp == JT // 2 - 1),
                            perf_mode=mybir.MatmulPerfMode.DoubleRow)
            for i in range(LT):
                nc.scalar.copy(xa_sb[:, i * d:(i + 1) * d], pc_acc[i][:])
            for ci, (o, w) in enumerate(chunks):
                nc.vector.tensor_copy(garow_sb[0:1, o:o + w], pga[ci][0:1, :])

        # ---- Phase D: transposes, gates, combine ----
        with tc.tile_pool(name="psD", bufs=1, space="PSUM") as psD, \
             tc.tile_pool(name="outD", bufs=2) as outp:
            pT = psD.tile([128, d], F32, tag="pT")
            # rowsum partition-reduce -> columns (cols 0:LT of pT)
            for i in range(LT):
                mm(pT[:, i:i + 1], rs_acc[:, i * 128:(i + 1) * 128],
                   ones_f32[:], i == 0, False, skip_group_check=True)
            # ga row -> columns (cols LT:2LT); gb arrives pre-columned
            for i in range(LT):
                mm(pT[:, LT + i:LT + i + 1],
                   garow_sb[0:1, i * 128:(i + 1) * 128],
                   ones_row[0:1, 0:1], False, i == LT - 1,
                   skip_group_check=True)
            for i in range(LT):
                # recip(rowsum + 1e-5)
                nc.vector.tensor_scalar_add(gate_sb[:, 3 * LT + i:3 * LT + i + 1],
                                            pT[:, i:i + 1], 1e-5)
                nc.vector.reciprocal(gate_sb[:, i:i + 1],
                                     gate_sb[:, 3 * LT + i:3 * LT + i + 1])
                # sig_a = sigmoid(ga + wa2x + ba)
                nc.vector.tensor_tensor(gate_sb[:, LT + i:LT + i + 1],
                                        pT[:, LT + i:LT + i + 1],
                                        stats_loc[:, i * 8 + 5:i * 8 + 6],
                                        op=ALU.add)
                nc.scalar.activation(gate_sb[:, LT + i:LT + i + 1],
                                     gate_sb[:, LT + i:LT + i + 1],
                                     AF.Sigmoid, bias=ba_sb[:])
            for i in range(LT):
                u_t = outp.tile([128, d], F32, tag="u")
                # u = sig_a * (x_a_raw * recip)
                nc.vector.tensor_scalar(u_t[:], xa_sb[:, i * d:(i + 1) * d],
                                        gate_sb[:, i:i + 1],
                                        gate_sb[:, LT + i:LT + i + 1],
                                        op0=ALU.mult, op1=ALU.mult)
                t_t = outp.tile([128, d], F32, tag="t")
                # y = sigmoid(t2 + u)
                nc.vector.tensor_tensor(t_t[:], t2_sb[:, i * d:(i + 1) * d],
                                        u_t[:], op=ALU.add)
                y_t = outp.tile([128, d], F32, tag="y")
                nc.scalar.activation(y_t[:], t_t[:], AF.Sigmoid)
                nc.sync.dma_start(out=out_dram[i * 128:(i + 1) * 128, :],
                                  in_=y_t[:])

    nc.compile()
    return nc


def make_r_matrices(W_sa, a_sa, W_gcnb, Wa, Wb, d):
    cols = np.zeros((d, 8), dtype=np.float32)
    cols[:, 0] = W_sa @ a_sa[0, :d]
    cols[:, 1] = W_sa @ a_sa[0, d:]
    # col 2 stays zero
    cols[:, 3] = Wa[0, :d]
    cols[:, 4] = Wb[0, :d]
    cols[:, 5] = Wa[0, d:]
    cols[:, 6] = Wb[0, d:]
    r_loc = np.ascontiguousarray(np.concatenate([W_sa, cols], axis=1))
    r_ext = np.ascontiguousarray(np.concatenate([W_gcnb, cols], axis=1))
    return r_loc.astype(np.float32), r_ext.astype(np.float32)


def _pack_lhsT(xm, KT):
    """[M*128, d] row-block -> [M, 128, KT*128] with tile (m,k) = block^T.

    Element (m, p, k*128+r) = xm[m*128+r, k*128+p], so a single [128, KT*128]
    DMA per m-tile lands k-blocks side by side in SBUF columns.
    """
    M = xm.shape[0] // 128
    return np.ascontiguousarray(
        xm.reshape(M, 128, KT, 128).transpose(0, 3, 2, 1).reshape(
            M, 128, KT * 128))


def make_shared_inputs(x, r_loc, r_ext, b_gcnb, n, d, np_bc):
    JT, KT = n // 128, d // 128
    xtf = _pack_lhsT(x, KT)
    return {
        "xtf": xtf.astype(np_bc),
        "rloc": r_loc.reshape(KT, 128, d + 8).astype(np.float32),
        "rext": r_ext.reshape(KT, 128, d + 8).astype(np_bc),
        "bbias": np.ascontiguousarray(
            np.broadcast_to(b_gcnb, (128, d))).astype(np.float32),
        "ident": np.eye(128, dtype=np.float32),
    }


def make_core_inputs(x, adj_a, adj_b, shared, n, d, nl, core, np_bc):
    JT, KT, LT = n // 128, d // 128, nl // 128
    rows = np.arange(core * nl, (core + 1) * nl)
    xtl = _pack_lhsT(x[rows], KT)
    adjat = np.ascontiguousarray(adj_a[rows].T).reshape(JT, 128, nl)
    adjbt = np.ascontiguousarray(adj_b[rows].T).reshape(JT, 128, nl)
    return {
        "xtl": xtl.astype(np.float32),
        "adjat": adjat.astype(np_bc),
        "adjbt": adjbt.astype(np_bc),
        **shared,
    }


_CACHE = {}


def _install_ntff_hook():
    """Dev-only: register the axon NTFF profile hook so trace=True works."""
    import sys
    import types
    try:
        from antenv import axon_hooks  # noqa: F401
        return
    except ImportError:
        pass
    import antenv
    mod = types.ModuleType("antenv.axon_hooks")
    _h = [None]
    mod.get_axon_ntff_profile_hook = lambda: _h[0]
    mod.set_axon_ntff_profile_hook = lambda hook: _h.__setitem__(0, hook)
    sys.modules["antenv.axon_hooks"] = mod
    antenv.axon_hooks = mod
    from trn_agent_boot.trn_boot import _ntff_profile_via_ctypes
    mod.set_axon_ntff_profile_hook(
        _ntff_profile_via_ctypes("/opt/axon/libaxon_pjrt.so"))


def kernel(x, adj_a, adj_b, W_sa, a_sa, W_gcnb, b_gcnb, Wa, ba, Wb, bb,
           _trace=False, _trace_kwargs=None):
    from concourse.bass_utils import run_bass_kernel_spmd
    if _trace:
        _install_ntff_hook()

    n, d = x.shape
    nl = n // N_CORES
    r_loc, r_ext = make_r_matrices(W_sa, a_sa, W_gcnb, Wa, Wb, d)

    key = (n, d, nl, float(ba[0]), float(bb[0]))
    if key not in _CACHE:
        _CACHE[key] = build_program(n, d, nl, float(ba[0]), float(bb[0]))
    nc = _CACHE[key]

    import ml_dtypes
    np_bc = ml_dtypes.bfloat16
    shared = make_shared_inputs(x, r_loc, r_ext, b_gcnb, n, d, np_bc)
    in_maps = [make_core_inputs(x, adj_a, adj_b, shared, n, d, nl, c, np_bc)
               for c in range(N_CORES)]
    res = run_bass_kernel_spmd(nc, in_maps, list(range(N_CORES)),
                               trace=_trace, **(_trace_kwargs or {}))
    out = np.empty((n, d), dtype=np.float32)
    for c in range(N_CORES):
        out[c * nl:(c + 1) * nl] = res.results[c]["out"]
    if _trace:
        kernel._last_results = res
    return out
